# revision 22
# baseline (speedup 1.0000x reference)
r"""Boson-sampling probability |Perm(A)|^2 via Glynn's formula on 8 Trainium2 cores.

Math
----
perm(A) = 2^(1-n) * sum_{d in {-1,+1}^n} (prod_i d_i) * prod_j (sum_i d_i A_ij), n=20.
Terms for d and -d are equal, so enumerate d_19 = -1 only and double.

Sign-bit allocation for the remaining 19 bits:
  bits 0..8   -> free axis f (512)       [same on every core]
  bits 9..15  -> partition axis p (128)  [same on every core]
  bits 16..18 -> core c (8)

Row vector V_j(p,f,c) = Cp_c[p,j] + Cf[f,j] with
  Cp_c[p,j] = sum_{i=9..15} d_i(p) A[i,j] + sum_{i=16..18} d_i(c) A[i,j] - A[19,j]
  Cf[f,j]   = sum_{i=0..8} d_i(f) A[i,j]

Split the j-product into groups GA=0..6, GB=7..13, GC=14..19; each group is
a rank-2^|G| bilinear form computed on TensorE as fp16 matmuls with PSUM
accumulation (contraction rows m = 2T+c interleave re/im).

Final layout (~19.4us HW, from ~20.3 baseline; measured facts driving it:
ring-1st transfers complete ~10.0/11-12us (Sync/ACT kick+stream), SWDGE-1st
~11.5-12.4, ring-2nd ~13.2 (+1.1us descriptor-fetch gap; every transfer's
16th sem increment straggles 0.2-1.2us behind the 15th on one slow DMA
engine); PE matmuls pipeline at ~427ns issue-to-issue regardless of
warm-up (a 3.2us continuous zero-matmul warm-up block did NOT raise the
p-state -- do not reintroduce); DVE occupancy ~340ns (fp16 2x TT), ~600ns
(PSUM-read 1x TT), ~610-680ns (STT, independent of operand space); ACT
ACTIVATE ~690ns; Pool TT ~1.2-1.4us and Pool cannot touch PSUM or run STT
(walrus rejects), so Pool carries no combine work; tensor_tensor_reduce
does not compile on this walrus build; fp8 e3m4 tables (exact per-row
pow2 rescale) measured rel err 0.217 -- the subset-product expansion
amplifies quantization noise ~50x, fp16 is required; SBUF-parameter
kernel I/O is not supported by the bass2jax/PJRT path):
- Tables split by re/im parts: the im lhsT is tiny (32KB vs 160KB per
  chunk), so A's re-part rides the ACT ring 1st slot and its im-part
  rides SWDGE; pgAre/pgAim complete ~0.5us earlier than v1's layout.
- DMA: Sync ring: C then Bre (+ the out store); ACT ring: Are;
  SWDGE: Aim then Bim. One sem per transfer (sem increments come from 16
  SDMA engines independently; FIFO order across transfers on a queue is
  NOT guaranteed per-engine, so thresholds cannot be stacked on one sem).
- Combine: ACT evicts sCre, sCim, sAim to fp16 and sBre scaled by 1/16
  (U_*PBre overflows fp16 unscaled; ~1.8e5 absmax). DVE spine: t1,t4
  (sC* x pgAre, 1x), t2,t3 (sC* x sAim, 2x), U_, P0, W_, P3 (2x), then
  STT reduces c2,c1 against pgBim (PSUM-direct; an STT's fp16 out tensor
  may saturate but accum_out accumulates the pre-cast fp32 product).
  ACT reduces P0,P3 (activation Copy + accum_out) into cols 0,3.
- Host: re = 16*o0 - o1, im = o2 + 16*o3; x2 for the d->-d symmetry.
"""

import numpy as np

N = 20
N_CORES = 8
F = 512           # free size (bits 0..8)
P = 128           # partitions (bits 9..15)
GA = list(range(0, 7))
GB = list(range(7, 14))
GC = list(range(14, 20))
WRE = P + F       # re-part chunk width: [lhsT_re | V]

_PROGRAM_CACHE = {}


def _signs(count, nbits):
    v = np.arange(count, dtype=np.int64)[:, None]
    return (((v >> np.arange(nbits)) & 1) * 2.0 - 1.0)  # (count, nbits) float64


def _subset_prods(C):
    """C: (nvals, g) complex128 -> (2^g, nvals); row T = prod_{k: bit k of T} C[:, k]."""
    out = np.ones((1, C.shape[0]), np.complex128)
    for k in range(C.shape[1]):
        out = np.concatenate([out, out * C[None, :, k]], axis=0)
    return out


def _pack_group(U, V):
    """Interleave re/im rows for the paired-contraction matmul layout.

    One shared V table streams through two matmuls; the re/im arithmetic is
    carried by two lhsT variants (contraction rows m = 2T + c):
      vtab[2T]   = Re V[T],  vtab[2T+1]   = Im V[T]
      lhs_re[2T] = Re U[T],  lhs_re[2T+1] = -Im U[T]   (-> PG_re)
      lhs_im[2T] = Im U[T],  lhs_im[2T+1] =  Re U[T]   (-> PG_im)
    """
    nT = U.shape[0]
    lre = np.empty((2 * nT, U.shape[1]), np.float32)
    lre[0::2] = U.real
    lre[1::2] = -U.imag
    lim = np.empty((2 * nT, U.shape[1]), np.float32)
    lim[0::2] = U.imag
    lim[1::2] = U.real
    vtab = np.empty((2 * nT, V.shape[1]), np.float32)
    vtab[0::2] = V.real
    vtab[1::2] = V.imag
    return lre, lim, vtab


def _build_core_tables(A, core):
    """Host tables for one core. A: (20,20) complex128.

    Per group, chunks of 128 contraction rows pack into a re-part
    [lhsT_re | vtab] (WRE cols per chunk) and an im-part [lhsT_im]
    (P cols per chunk); the vtab is shared by the re and im matmuls.
      tabC   [128, 2P+F]   single chunk [lre | lim | vtab]
      tabAre [128, 2*WRE]  chunk0 [lre|vtab], chunk1 [lre|vtab]
      tabAim [128, 2*P]    chunk0 lim, chunk1 lim
      tabBre / tabBim      same as A
    """
    f_signs = _signs(F, 9)
    p_signs = _signs(P, 7)
    c_signs = _signs(N_CORES, 3)
    par_f = np.prod(f_signs, axis=1)
    par_p = np.prod(p_signs, axis=1)
    par_c = np.prod(c_signs[core])

    Cf = f_signs @ A[0:9, :]                                         # (512, 20)
    Cp = p_signs @ A[9:16, :] + (c_signs[core] @ A[16:19, :] - A[19, :])[None, :]

    parts = {}
    for name, G in (("A", GA), ("B", GB), ("C", GC)):
        U = _subset_prods(Cp[:, G])          # (2^g, 128)
        VV = _subset_prods(Cf[:, G])         # (2^g, 512)
        V = VV[::-1]                         # complement subset: T -> 2^g-1-T
        if name == "A":
            # fold full parity: par_p(p) * par_f(f) * par_c * (-1 for d19)
            U = U * (par_p[None, :] * (-par_c))
            V = V * par_f[None, :]
        lre, lim, vtab = _pack_group(U, V)
        # NOTE: fp8 e3m4 tables (with exact per-row power-of-2 rescaling)
        # were measured at rel err 0.217 -- the subset-product expansion
        # cancels heavily (row terms ~350 summing to O(1)), amplifying
        # quantization noise ~50x. fp16 is required.
        parts[name] = tuple(x.astype(np.float16) for x in (lre, lim, vtab))

    lre, lim, vtab = parts["C"]
    tabs = {"tabC": np.ascontiguousarray(
        np.concatenate([lre, lim, vtab], axis=1))}
    for name in ("A", "B"):
        lre, lim, vtab = parts[name]
        re_chunks, im_chunks = [], []
        for k in range(2):
            sl = slice(k * 128, (k + 1) * 128)
            re_chunks += [lre[sl], vtab[sl]]
            im_chunks += [lim[sl]]
        tabs["tab%sre" % name] = np.ascontiguousarray(
            np.concatenate(re_chunks, axis=1))     # (128, 2*WRE)
        tabs["tab%sim" % name] = np.ascontiguousarray(
            np.concatenate(im_chunks, axis=1))     # (128, 2*P)
    return tabs


def _build_program():
    if "prog" in _PROGRAM_CACHE:
        return _PROGRAM_CACHE["prog"]

    from contextlib import ExitStack
    from concourse import bass, mybir

    f32 = mybir.dt.float32
    f16 = mybir.dt.float16
    mul = mybir.AluOpType.mult
    cp = mybir.ActivationFunctionType.Copy
    nc = bass.Bass()

    dram = {
        "C": nc.declare_dram_parameter("tabC", [128, 2 * P + F], f16, isOutput=False),
        "Are": nc.declare_dram_parameter("tabAre", [128, 2 * WRE], f16, isOutput=False),
        "Aim": nc.declare_dram_parameter("tabAim", [128, 2 * P], f16, isOutput=False),
        "Bre": nc.declare_dram_parameter("tabBre", [128, 2 * WRE], f16, isOutput=False),
        "Bim": nc.declare_dram_parameter("tabBim", [128, 2 * P], f16, isOutput=False),
    }
    out_dram = nc.declare_dram_parameter("out", [P, 4], f32, isOutput=True)

    es = ExitStack()
    with es:
        dma_c = es.enter_context(nc.semaphore("dma_c"))
        dma_are = es.enter_context(nc.semaphore("dma_are"))
        dma_aim = es.enter_context(nc.semaphore("dma_aim"))
        dma_bre = es.enter_context(nc.semaphore("dma_bre"))
        dma_bim = es.enter_context(nc.semaphore("dma_bim"))
        pe_sem = es.enter_context(nc.semaphore("pe_sem"))
        act_sem = es.enter_context(nc.semaphore("act_sem"))
        dve_sem = es.enter_context(nc.semaphore("dve_sem"))

        sb = {k: es.enter_context(nc.sbuf_tensor("sb_" + k, list(d.shape), f16))
              for k, d in dram.items()}
        names = ["sCre", "sCim", "sAim", "sBre",
                 "t1", "t2", "t3", "t4", "U_", "W_", "P0", "P3", "scr", "scrA"]
        wt = {n: es.enter_context(nc.sbuf_tensor(n, [P, F], f16)) for n in names}
        out_t = es.enter_context(nc.sbuf_tensor("out_t", [P, 4], f32))
        dummy = es.enter_context(nc.sbuf_tensor("actwarm", [P, 2], f32))
        pg = {}
        for g in ("A", "B", "C"):
            for comp in ("re", "im"):
                pg[g + comp] = es.enter_context(
                    nc.psum_tensor("pg" + g + comp, [P, F], f32))

        # --- Sync: C then Bre loads; final out store ---
        sync = nc.sync
        sync.dma_start(sb["C"][:, :], dram["C"][:, :]).then_inc(dma_c, 16)
        sync.dma_start(sb["Bre"][:, :], dram["Bre"][:, :]).then_inc(dma_bre, 16)
        sync.wait_ge(dve_sem, 10)
        sync.wait_ge(act_sem, 6)
        sync.dma_start(out_dram[:], out_t[:, :]).then_inc(dma_c, 16)

        # --- ACT: Are load; evictions sCre, sCim, sBre/16; reduces c0, c3 ---
        act = nc.scalar
        act.dma_start(sb["Are"][:, :], dram["Are"][:, :]).then_inc(dma_are, 16)
        # trigger walrus's activation-table load during the DMA window
        act.copy(dummy[:, 1:2], dummy[:, 0:1])
        act.wait_ge(pe_sem, 1)
        act.copy(wt["sCre"][:, :], pg["Cre"][:, :]).then_inc(act_sem, 1)
        act.wait_ge(pe_sem, 2)
        act.copy(wt["sCim"][:, :], pg["Cim"][:, :]).then_inc(act_sem, 1)
        act.wait_ge(pe_sem, 6)
        act.copy(wt["sAim"][:, :], pg["Aim"][:, :]).then_inc(act_sem, 1)
        # x1/16: U_*PBre / W_*PBre products overflow fp16 unscaled
        act.wait_ge(pe_sem, 8)
        act.mul(wt["sBre"][:, :], pg["Bre"][:, :], 0.0625).then_inc(act_sem, 1)
        act.wait_ge(dve_sem, 6)         # P0 = U_*sBre ready
        act.activation(wt["scrA"][:, :], wt["P0"][:, :], cp,
                       accum_out=out_t[:, 0:1]).then_inc(act_sem, 1)
        act.wait_ge(dve_sem, 8)         # P3 = W_*sBre ready
        act.activation(wt["scrA"][:, :], wt["P3"][:, :], cp,
                       accum_out=out_t[:, 3:4]).then_inc(act_sem, 1)

        # --- GpSimd: SWDGE loads Aim then Bim ---
        gp = nc.gpsimd
        gp.dma_start(sb["Aim"][:, :], dram["Aim"][:, :]).then_inc(dma_aim, 16)
        gp.dma_start(sb["Bim"][:, :], dram["Bim"][:, :]).then_inc(dma_bim, 16)

        # --- PE: C (2), A (4), B (4) ---
        pe = nc.tensor
        # pe_sem: pgCre 1, pgCim 2, pgAre 4, pgAim 6, pgBre 8, pgBim 10

        def mmc(comp, wait=None, thr=0):
            if wait is not None:
                pe.wait_ge(wait, thr)
            lo = 0 if comp == "re" else P
            pe.matmul(pg["C" + comp][:, :], sb["C"][:, lo:lo + P],
                      sb["C"][:, 2 * P:2 * P + F],
                      start=True, stop=True).then_inc(pe_sem, 1)

        def mm2(g, comp, k, wait=None, thr=0):
            if wait is not None:
                pe.wait_ge(wait, thr)
            if comp == "re":
                lhs = sb[g + "re"][:, k * WRE:k * WRE + P]
            else:
                lhs = sb[g + "im"][:, k * P:(k + 1) * P]
            rhs = sb[g + "re"][:, k * WRE + P:(k + 1) * WRE]
            pe.matmul(pg[g + comp][:, :], lhs, rhs,
                      start=(k == 0), stop=(k == 1)).then_inc(pe_sem, 1)

        mmc("re", dma_c, 16)
        mmc("im")
        mm2("A", "re", 0, dma_are, 16)
        mm2("A", "re", 1)
        mm2("A", "im", 0, dma_aim, 16)
        mm2("A", "im", 1)
        mm2("B", "re", 0, dma_bre, 16)
        mm2("B", "re", 1)
        mm2("B", "im", 0, dma_bim, 16)
        mm2("B", "im", 1)

        # --- DVE: combine spine ---
        v = nc.vector
        # dve_sem: t1 1, t4 2, t2 3, t3 4, U_ 5, P0 6, W_ 7, P3 8, c2 9, c1 10
        v.wait_ge(act_sem, 1)
        v.wait_ge(pe_sem, 4)
        v.tensor_mul(wt["t1"][:, :], wt["sCre"][:, :], pg["Are"][:, :]).then_inc(dve_sem, 1)
        v.wait_ge(act_sem, 2)
        v.tensor_mul(wt["t4"][:, :], wt["sCim"][:, :], pg["Are"][:, :]).then_inc(dve_sem, 1)
        v.wait_ge(act_sem, 3)
        v.tensor_mul(wt["t2"][:, :], wt["sCim"][:, :], wt["sAim"][:, :]).then_inc(dve_sem, 1)
        v.tensor_mul(wt["t3"][:, :], wt["sCre"][:, :], wt["sAim"][:, :]).then_inc(dve_sem, 1)
        v.tensor_sub(wt["U_"][:, :], wt["t1"][:, :], wt["t2"][:, :]).then_inc(dve_sem, 1)
        v.wait_ge(act_sem, 4)
        v.tensor_mul(wt["P0"][:, :], wt["U_"][:, :], wt["sBre"][:, :]).then_inc(dve_sem, 1)
        v.tensor_add(wt["W_"][:, :], wt["t3"][:, :], wt["t4"][:, :]).then_inc(dve_sem, 1)
        v.tensor_mul(wt["P3"][:, :], wt["W_"][:, :], wt["sBre"][:, :]).then_inc(dve_sem, 1)
        # out cols: 0 = sum U*sBre (ACT), 1 = sum W*PBim, 2 = sum U*PBim,
        # 3 = sum W*sBre (ACT); host: re = 16*o0 - o1, im = o2 + 16*o3.
        # (tensor_tensor_reduce does not compile on this walrus build.)
        v.wait_ge(pe_sem, 10)
        v.scalar_tensor_tensor(
            wt["scr"][:, :], wt["U_"][:, :], 1.0, pg["Bim"][:, :],
            mul, mul, accum_out=out_t[:, 2:3]).then_inc(dve_sem, 1)
        v.scalar_tensor_tensor(
            wt["scr"][:, :], wt["W_"][:, :], 1.0, pg["Bim"][:, :],
            mul, mul, accum_out=out_t[:, 1:2]).then_inc(dve_sem, 1)

    nc.finalize()
    _PROGRAM_CACHE["prog"] = nc
    return nc


def kernel(A_real, A_imag, _collect=None):
    from concourse.bass_utils import run_bass_kernel_spmd

    A = np.asarray(A_real, np.float64) + 1j * np.asarray(A_imag, np.float64)
    nc = _build_program()
    in_maps = [_build_core_tables(A, c) for c in range(N_CORES)]

    kwargs = dict(_collect or {})
    res = run_bass_kernel_spmd(nc, in_maps, core_ids=list(range(N_CORES)), **kwargs)
    if _collect is not None:
        _collect["results"] = res

    total = np.complex128(0)
    for r in res.results:
        o = np.asarray(r["out"], np.float64)
        total += (16.0 * o[:, 0] - o[:, 1]).sum() + 1j * (o[:, 2] + 16.0 * o[:, 3]).sum()

    perm = total * 2.0 * (2.0 ** (1 - N))
    ans = (perm.conjugate() * perm).real
    return np.asarray(ans, np.float32)


# revision 24
# speedup vs baseline: 1.0978x; 1.0978x over previous
r"""Boson-sampling probability |Perm(A)|^2 via Glynn's formula on 8 Trainium2 cores.

Math
----
perm(A) = 2^(1-n) * sum_{d in {-1,+1}^n} (prod_i d_i) * prod_j (sum_i d_i A_ij), n=20.
Terms for d and -d are equal, so enumerate d_19 = -1 only and double.

Sign-bit allocation for the remaining 19 bits:
  bits 0..8   -> free axis f (512)       [same on every core]
  bits 9..15  -> partition axis p (128)  [same on every core]
  bits 16..18 -> core c (8)

Row vector V_j(p,f,c) = Cp_c[p,j] + Cf[f,j] with
  Cp_c[p,j] = sum_{i=9..15} d_i(p) A[i,j] + sum_{i=16..18} d_i(c) A[i,j] - A[19,j]
  Cf[f,j]   = sum_{i=0..8} d_i(f) A[i,j]

Split the j-product into groups GA=0..6, GB=7..13, GC=14..19; each group is
a rank-2^|G| bilinear form computed on TensorE as fp16 matmuls with PSUM
accumulation (contraction rows m = 2T+c interleave re/im).

Final layout (~19.4us HW, from ~20.3 baseline; measured facts driving it:
ring-1st transfers complete ~10.0/11-12us (Sync/ACT kick+stream), SWDGE-1st
~11.5-12.4, ring-2nd ~13.2 (+1.1us descriptor-fetch gap; every transfer's
16th sem increment straggles 0.2-1.2us behind the 15th on one slow DMA
engine); PE matmuls pipeline at ~427ns issue-to-issue regardless of
warm-up (a 3.2us continuous zero-matmul warm-up block did NOT raise the
p-state -- do not reintroduce); DVE occupancy ~340ns (fp16 2x TT), ~600ns
(PSUM-read 1x TT), ~610-680ns (STT, independent of operand space); ACT
ACTIVATE ~690ns; Pool TT ~1.2-1.4us and Pool cannot touch PSUM or run STT
(walrus rejects), so Pool carries no combine work; tensor_tensor_reduce
does not compile on this walrus build; fp8 e3m4 tables (exact per-row
pow2 rescale) measured rel err 0.217 -- the subset-product expansion
amplifies quantization noise ~50x, fp16 is required; SBUF-parameter
kernel I/O is not supported by the bass2jax/PJRT path):
- Tables split by re/im parts: the im lhsT is tiny (32KB vs 160KB per
  chunk), so A's re-part rides the ACT ring 1st slot and its im-part
  rides SWDGE; pgAre/pgAim complete ~0.5us earlier than v1's layout.
- DMA: Sync ring: C then Bre (+ the out store); ACT ring: Are;
  SWDGE: Aim then Bim. One sem per transfer (sem increments come from 16
  SDMA engines independently; FIFO order across transfers on a queue is
  NOT guaranteed per-engine, so thresholds cannot be stacked on one sem).
- Combine: ACT evicts sCre, sCim, sAim to fp16 and sBre scaled by 1/16
  (U_*PBre overflows fp16 unscaled; ~1.8e5 absmax). DVE spine: t1,t4
  (sC* x pgAre, 1x), t2,t3 (sC* x sAim, 2x), U_, P0, W_, P3 (2x), then
  STT reduces c2,c1 against pgBim (PSUM-direct; an STT's fp16 out tensor
  may saturate but accum_out accumulates the pre-cast fp32 product).
  ACT reduces P0,P3 (activation Copy + accum_out) into cols 0,3.
- Host: re = 16*o0 - o1, im = o2 + 16*o3; x2 for the d->-d symmetry.
"""

import numpy as np

N = 20
N_CORES = 8
F = 512           # free size (bits 0..8)
P = 128           # partitions (bits 9..15)
GA = list(range(0, 7))
GB = list(range(7, 14))
GC = list(range(14, 20))
WRE = P + F       # re-part chunk width: [lhsT_re | V]

_PROGRAM_CACHE = {}


def _signs(count, nbits):
    v = np.arange(count, dtype=np.int64)[:, None]
    return (((v >> np.arange(nbits)) & 1) * 2.0 - 1.0)  # (count, nbits) float64


def _subset_prods(C):
    """C: (nvals, g) complex128 -> (2^g, nvals); row T = prod_{k: bit k of T} C[:, k]."""
    out = np.ones((1, C.shape[0]), np.complex128)
    for k in range(C.shape[1]):
        out = np.concatenate([out, out * C[None, :, k]], axis=0)
    return out


def _pack_group(U, V):
    """Interleave re/im rows for the paired-contraction matmul layout.

    One shared V table streams through two matmuls; the re/im arithmetic is
    carried by two lhsT variants (contraction rows m = 2T + c):
      vtab[2T]   = Re V[T],  vtab[2T+1]   = Im V[T]
      lhs_re[2T] = Re U[T],  lhs_re[2T+1] = -Im U[T]   (-> PG_re)
      lhs_im[2T] = Im U[T],  lhs_im[2T+1] =  Re U[T]   (-> PG_im)
    """
    nT = U.shape[0]
    lre = np.empty((2 * nT, U.shape[1]), np.float32)
    lre[0::2] = U.real
    lre[1::2] = -U.imag
    lim = np.empty((2 * nT, U.shape[1]), np.float32)
    lim[0::2] = U.imag
    lim[1::2] = U.real
    vtab = np.empty((2 * nT, V.shape[1]), np.float32)
    vtab[0::2] = V.real
    vtab[1::2] = V.imag
    return lre, lim, vtab


def _build_core_tables(A, core):
    """Host tables for one core. A: (20,20) complex128.

    Per group, chunks of 128 contraction rows pack into a re-part
    [lhsT_re | vtab] (WRE cols per chunk) and an im-part [lhsT_im]
    (P cols per chunk); the vtab is shared by the re and im matmuls.
      tabC   [128, 2P+F]   single chunk [lre | lim | vtab]
      tabAre [128, 2*WRE]  chunk0 [lre|vtab], chunk1 [lre|vtab]
      tabAim [128, 2*P]    chunk0 lim, chunk1 lim
      tabBre / tabBim      same as A
    """
    f_signs = _signs(F, 9)
    p_signs = _signs(P, 7)
    c_signs = _signs(N_CORES, 3)
    par_f = np.prod(f_signs, axis=1)
    par_p = np.prod(p_signs, axis=1)
    par_c = np.prod(c_signs[core])

    Cf = f_signs @ A[0:9, :]                                         # (512, 20)
    Cp = p_signs @ A[9:16, :] + (c_signs[core] @ A[16:19, :] - A[19, :])[None, :]

    parts = {}
    for name, G in (("A", GA), ("B", GB), ("C", GC)):
        U = _subset_prods(Cp[:, G])          # (2^g, 128)
        VV = _subset_prods(Cf[:, G])         # (2^g, 512)
        V = VV[::-1]                         # complement subset: T -> 2^g-1-T
        if name == "A":
            # fold full parity: par_p(p) * par_f(f) * par_c * (-1 for d19)
            U = U * (par_p[None, :] * (-par_c))
            V = V * par_f[None, :]
        lre, lim, vtab = _pack_group(U, V)
        # NOTE: fp8 e3m4 tables (with exact per-row power-of-2 rescaling)
        # were measured at rel err 0.217 -- the subset-product expansion
        # cancels heavily (row terms ~350 summing to O(1)), amplifying
        # quantization noise ~50x. fp16 is required.
        parts[name] = tuple(x.astype(np.float16) for x in (lre, lim, vtab))

    lre, lim, vtab = parts["C"]
    tabs = {"tabC": np.ascontiguousarray(
        np.concatenate([lre, lim, vtab], axis=1))}
    for name in ("A", "B"):
        lre, lim, vtab = parts[name]
        re_chunks, im_chunks = [], []
        for k in range(2):
            sl = slice(k * 128, (k + 1) * 128)
            re_chunks += [lre[sl], vtab[sl]]
            im_chunks += [lim[sl]]
        tabs["tab%sre" % name] = np.ascontiguousarray(
            np.concatenate(re_chunks, axis=1))     # (128, 2*WRE)
        tabs["tab%sim" % name] = np.ascontiguousarray(
            np.concatenate(im_chunks, axis=1))     # (128, 2*P)
    return tabs


def _build_program():
    if "prog" in _PROGRAM_CACHE:
        return _PROGRAM_CACHE["prog"]

    from contextlib import ExitStack
    from concourse import bass, mybir

    f32 = mybir.dt.float32
    f16 = mybir.dt.float16
    mul = mybir.AluOpType.mult
    cp = mybir.ActivationFunctionType.Copy
    nc = bass.Bass()

    dram = {
        "C": nc.declare_dram_parameter("tabC", [128, 2 * P + F], f16, isOutput=False),
        "Are": nc.declare_dram_parameter("tabAre", [128, 2 * WRE], f16, isOutput=False),
        "Aim": nc.declare_dram_parameter("tabAim", [128, 2 * P], f16, isOutput=False),
        "Bre": nc.declare_dram_parameter("tabBre", [128, 2 * WRE], f16, isOutput=False),
        "Bim": nc.declare_dram_parameter("tabBim", [128, 2 * P], f16, isOutput=False),
    }
    out_dram = nc.declare_dram_parameter("out", [P, 4], f32, isOutput=True)

    es = ExitStack()
    with es:
        dma_c = es.enter_context(nc.semaphore("dma_c"))
        dma_are = es.enter_context(nc.semaphore("dma_are"))
        dma_aim = es.enter_context(nc.semaphore("dma_aim"))
        dma_bre = es.enter_context(nc.semaphore("dma_bre"))
        dma_bim = es.enter_context(nc.semaphore("dma_bim"))
        pe_sem = es.enter_context(nc.semaphore("pe_sem"))
        act_sem = es.enter_context(nc.semaphore("act_sem"))
        dve_sem = es.enter_context(nc.semaphore("dve_sem"))

        sb = {k: es.enter_context(nc.sbuf_tensor("sb_" + k, list(d.shape), f16))
              for k, d in dram.items()}
        names = ["sCre", "sCim", "sAim", "sBre",
                 "t1", "t2", "t3", "t4", "U_", "W_", "P0", "P3", "scr", "scrA"]
        wt = {n: es.enter_context(nc.sbuf_tensor(n, [P, F], f16)) for n in names}
        out_t = es.enter_context(nc.sbuf_tensor("out_t", [P, 4], f32))
        dummy = es.enter_context(nc.sbuf_tensor("actwarm", [P, 2], f32))
        pg = {}
        for g in ("A", "B", "C"):
            for comp in ("re", "im"):
                pg[g + comp] = es.enter_context(
                    nc.psum_tensor("pg" + g + comp, [P, F], f32))

        # --- Sync: C then Bre loads; final out store ---
        sync = nc.sync
        sync.dma_start(sb["C"][:, :], dram["C"][:, :]).then_inc(dma_c, 16)
        sync.dma_start(sb["Bre"][:, :], dram["Bre"][:, :]).then_inc(dma_bre, 16)
        sync.wait_ge(dve_sem, 10)
        sync.wait_ge(act_sem, 6)
        sync.dma_start(out_dram[:], out_t[:, :]).then_inc(dma_c, 16)

        # --- ACT: Are load; evictions sCre, sCim, sBre/16; reduces c0, c3 ---
        act = nc.scalar
        act.dma_start(sb["Are"][:, :], dram["Are"][:, :]).then_inc(dma_are, 16)
        # trigger walrus's activation-table load during the DMA window
        act.copy(dummy[:, 1:2], dummy[:, 0:1])
        act.wait_ge(pe_sem, 1)
        act.copy(wt["sCre"][:, :], pg["Cre"][:, :]).then_inc(act_sem, 1)
        act.wait_ge(pe_sem, 2)
        act.copy(wt["sCim"][:, :], pg["Cim"][:, :]).then_inc(act_sem, 1)
        act.wait_ge(pe_sem, 6)
        act.copy(wt["sAim"][:, :], pg["Aim"][:, :]).then_inc(act_sem, 1)
        # x1/16: U_*PBre / W_*PBre products overflow fp16 unscaled
        act.wait_ge(pe_sem, 8)
        act.mul(wt["sBre"][:, :], pg["Bre"][:, :], 0.0625).then_inc(act_sem, 1)
        act.wait_ge(dve_sem, 6)         # P0 = U_*sBre ready
        act.activation(wt["scrA"][:, :], wt["P0"][:, :], cp,
                       accum_out=out_t[:, 0:1]).then_inc(act_sem, 1)
        act.wait_ge(dve_sem, 8)         # P3 = W_*sBre ready
        act.activation(wt["scrA"][:, :], wt["P3"][:, :], cp,
                       accum_out=out_t[:, 3:4]).then_inc(act_sem, 1)

        # --- GpSimd: SWDGE loads Aim then Bim ---
        gp = nc.gpsimd
        gp.dma_start(sb["Aim"][:, :], dram["Aim"][:, :]).then_inc(dma_aim, 16)
        gp.dma_start(sb["Bim"][:, :], dram["Bim"][:, :]).then_inc(dma_bim, 16)

        # --- PE: C (2), A (4), B (4) ---
        pe = nc.tensor
        # pe_sem: pgCre 1, pgCim 2, pgAre 4, pgAim 6, pgBre 8, pgBim 10

        def mmc(comp, wait=None, thr=0):
            if wait is not None:
                pe.wait_ge(wait, thr)
            lo = 0 if comp == "re" else P
            pe.matmul(pg["C" + comp][:, :], sb["C"][:, lo:lo + P],
                      sb["C"][:, 2 * P:2 * P + F],
                      start=True, stop=True).then_inc(pe_sem, 1)

        def mm2(g, comp, k, wait=None, thr=0):
            if wait is not None:
                pe.wait_ge(wait, thr)
            if comp == "re":
                lhs = sb[g + "re"][:, k * WRE:k * WRE + P]
            else:
                lhs = sb[g + "im"][:, k * P:(k + 1) * P]
            rhs = sb[g + "re"][:, k * WRE + P:(k + 1) * WRE]
            pe.matmul(pg[g + comp][:, :], lhs, rhs,
                      start=(k == 0), stop=(k == 1)).then_inc(pe_sem, 1)

        mmc("re", dma_c, 16)
        mmc("im")
        mm2("A", "re", 0, dma_are, 16)
        mm2("A", "re", 1)
        mm2("A", "im", 0, dma_aim, 16)
        mm2("A", "im", 1)
        mm2("B", "re", 0, dma_bre, 16)
        mm2("B", "re", 1)
        mm2("B", "im", 0, dma_bim, 16)
        mm2("B", "im", 1)

        # --- DVE: combine spine ---
        v = nc.vector
        # dve_sem: t1 1, t4 2, t2 3, t3 4, U_ 5, P0 6, W_ 7, P3 8, c2 9, c1 10
        v.wait_ge(act_sem, 1)
        v.wait_ge(pe_sem, 4)
        v.tensor_mul(wt["t1"][:, :], wt["sCre"][:, :], pg["Are"][:, :]).then_inc(dve_sem, 1)
        v.wait_ge(act_sem, 2)
        v.tensor_mul(wt["t4"][:, :], wt["sCim"][:, :], pg["Are"][:, :]).then_inc(dve_sem, 1)
        v.wait_ge(act_sem, 3)
        v.tensor_mul(wt["t2"][:, :], wt["sCim"][:, :], wt["sAim"][:, :]).then_inc(dve_sem, 1)
        v.tensor_mul(wt["t3"][:, :], wt["sCre"][:, :], wt["sAim"][:, :]).then_inc(dve_sem, 1)
        v.tensor_sub(wt["U_"][:, :], wt["t1"][:, :], wt["t2"][:, :]).then_inc(dve_sem, 1)
        v.wait_ge(act_sem, 4)
        v.tensor_mul(wt["P0"][:, :], wt["U_"][:, :], wt["sBre"][:, :]).then_inc(dve_sem, 1)
        v.tensor_add(wt["W_"][:, :], wt["t3"][:, :], wt["t4"][:, :]).then_inc(dve_sem, 1)
        v.tensor_mul(wt["P3"][:, :], wt["W_"][:, :], wt["sBre"][:, :]).then_inc(dve_sem, 1)
        # out cols: 0 = sum U*sBre (ACT), 1 = sum W*PBim, 2 = sum U*PBim,
        # 3 = sum W*sBre (ACT); host: re = 16*o0 - o1, im = o2 + 16*o3.
        # (tensor_tensor_reduce does not compile on this walrus build.)
        v.wait_ge(pe_sem, 10)
        v.scalar_tensor_tensor(
            wt["scr"][:, :], wt["U_"][:, :], 1.0, pg["Bim"][:, :],
            mul, mul, accum_out=out_t[:, 2:3]).then_inc(dve_sem, 1)
        v.scalar_tensor_tensor(
            wt["scr"][:, :], wt["W_"][:, :], 1.0, pg["Bim"][:, :],
            mul, mul, accum_out=out_t[:, 1:2]).then_inc(dve_sem, 1)

    nc.finalize()
    _PROGRAM_CACHE["prog"] = nc
    return nc


def kernel(A_real, A_imag, _collect=None):
    from concourse.bass_utils import run_bass_kernel_spmd

    A = np.asarray(A_real, np.float64) + 1j * np.asarray(A_imag, np.float64)
    nc = _build_program()
    in_maps = [_build_core_tables(A, c) for c in range(N_CORES)]

    kwargs = dict(_collect or {})
    res = run_bass_kernel_spmd(nc, in_maps, core_ids=list(range(N_CORES)), **kwargs)
    if _collect is not None:
        _collect["results"] = res

    total = np.complex128(0)
    for r in res.results:
        o = np.asarray(r["out"], np.float64)
        total += (16.0 * o[:, 0] - o[:, 1]).sum() + 1j * (o[:, 2] + 16.0 * o[:, 3]).sum()

    perm = total * 2.0 * (2.0 ** (1 - N))
    ans = (perm.conjugate() * perm).real
    return np.asarray(ans, np.float32)


# revision 28
# speedup vs baseline: 1.1231x; 1.0230x over previous
r"""Boson-sampling probability |Perm(A)|^2 via Glynn's formula on 8 Trainium2 cores.

Math
----
perm(A) = 2^(1-n) * sum_{d in {-1,+1}^n} (prod_i d_i) * prod_j (sum_i d_i A_ij), n=20.
Terms for d and -d are equal, so enumerate d_19 = -1 only and double.

Sign-bit allocation for the remaining 19 bits:
  bits 0..8   -> free axis f (512)       [same on every core]
  bits 9..15  -> partition axis p (128)  [same on every core]
  bits 16..18 -> core c (8)

Row vector V_j(p,f,c) = Cp_c[p,j] + Cf[f,j] with
  Cp_c[p,j] = sum_{i=9..15} d_i(p) A[i,j] + sum_{i=16..18} d_i(c) A[i,j] - A[19,j]
  Cf[f,j]   = sum_{i=0..8} d_i(f) A[i,j]

Split the j-product into groups GA=0..6, GB=7..13, GC=14..19; each group is
a rank-2^|G| bilinear form computed on TensorE as fp16 matmuls with PSUM
accumulation (contraction rows m = 2T+c interleave re/im).

Final layout (~19.4us HW, from ~20.3 baseline; measured facts driving it:
ring-1st transfers complete ~10.0/11-12us (Sync/ACT kick+stream), SWDGE-1st
~11.5-12.4, ring-2nd ~13.2 (+1.1us descriptor-fetch gap; every transfer's
16th sem increment straggles 0.2-1.2us behind the 15th on one slow DMA
engine); PE matmuls pipeline at ~427ns issue-to-issue regardless of
warm-up (a 3.2us continuous zero-matmul warm-up block did NOT raise the
p-state -- do not reintroduce); DVE occupancy ~340ns (fp16 2x TT), ~600ns
(PSUM-read 1x TT), ~610-680ns (STT, independent of operand space); ACT
ACTIVATE ~690ns; Pool TT ~1.2-1.4us and Pool cannot touch PSUM or run STT
(walrus rejects), so Pool carries no combine work; tensor_tensor_reduce
does not compile on this walrus build; fp8 e3m4 tables (exact per-row
pow2 rescale) measured rel err 0.217 -- the subset-product expansion
amplifies quantization noise ~50x, fp16 is required; SBUF-parameter
kernel I/O is not supported by the bass2jax/PJRT path):
- Tables split by re/im parts: the im lhsT is tiny (32KB vs 160KB per
  chunk), so A's re-part rides the ACT ring 1st slot and its im-part
  rides SWDGE; pgAre/pgAim complete ~0.5us earlier than v1's layout.
- DMA: Sync ring: C then Bre (+ the out store); ACT ring: Are;
  SWDGE: Aim then Bim. One sem per transfer (sem increments come from 16
  SDMA engines independently; FIFO order across transfers on a queue is
  NOT guaranteed per-engine, so thresholds cannot be stacked on one sem).
- Combine: ACT evicts sCre, sCim, sAim to fp16 and sBre scaled by 1/16
  (U_*PBre overflows fp16 unscaled; ~1.8e5 absmax). DVE spine: t1,t4
  (sC* x pgAre, 1x), t2,t3 (sC* x sAim, 2x), U_, P0, W_, P3 (2x), then
  STT reduces c2,c1 against pgBim (PSUM-direct; an STT's fp16 out tensor
  may saturate but accum_out accumulates the pre-cast fp32 product).
  ACT reduces P0,P3 (activation Copy + accum_out) into cols 0,3.
- Host: re = 16*o0 - o1, im = o2 + 16*o3; x2 for the d->-d symmetry.
"""

import numpy as np

N = 20
N_CORES = 8
F = 512           # free size (bits 0..8)
P = 128           # partitions (bits 9..15)
GA = list(range(0, 7))
GB = list(range(7, 14))
GC = list(range(14, 20))
WRE = P + F       # re-part chunk width: [lhsT_re | V]

_PROGRAM_CACHE = {}


def _signs(count, nbits):
    v = np.arange(count, dtype=np.int64)[:, None]
    return (((v >> np.arange(nbits)) & 1) * 2.0 - 1.0)  # (count, nbits) float64


def _subset_prods(C):
    """C: (nvals, g) complex128 -> (2^g, nvals); row T = prod_{k: bit k of T} C[:, k]."""
    out = np.ones((1, C.shape[0]), np.complex128)
    for k in range(C.shape[1]):
        out = np.concatenate([out, out * C[None, :, k]], axis=0)
    return out


def _pack_group(U, V):
    """Interleave re/im rows for the paired-contraction matmul layout.

    One shared V table streams through two matmuls; the re/im arithmetic is
    carried by two lhsT variants (contraction rows m = 2T + c):
      vtab[2T]   = Re V[T],  vtab[2T+1]   = Im V[T]
      lhs_re[2T] = Re U[T],  lhs_re[2T+1] = -Im U[T]   (-> PG_re)
      lhs_im[2T] = Im U[T],  lhs_im[2T+1] =  Re U[T]   (-> PG_im)
    """
    nT = U.shape[0]
    lre = np.empty((2 * nT, U.shape[1]), np.float32)
    lre[0::2] = U.real
    lre[1::2] = -U.imag
    lim = np.empty((2 * nT, U.shape[1]), np.float32)
    lim[0::2] = U.imag
    lim[1::2] = U.real
    vtab = np.empty((2 * nT, V.shape[1]), np.float32)
    vtab[0::2] = V.real
    vtab[1::2] = V.imag
    return lre, lim, vtab


def _build_core_tables(A, core):
    """Host tables for one core. A: (20,20) complex128.

    Per group, chunks of 128 contraction rows pack into a re-part
    [lhsT_re | vtab] (WRE cols per chunk) and an im-part [lhsT_im]
    (P cols per chunk); the vtab is shared by the re and im matmuls.
      tabC   [128, 2P+F]   single chunk [lre | lim | vtab]
      tabAre [128, 2*WRE]  chunk0 [lre|vtab], chunk1 [lre|vtab]
      tabAim [128, 2*P]    chunk0 lim, chunk1 lim
      tabBre / tabBim      same as A
    """
    f_signs = _signs(F, 9)
    p_signs = _signs(P, 7)
    c_signs = _signs(N_CORES, 3)
    par_f = np.prod(f_signs, axis=1)
    par_p = np.prod(p_signs, axis=1)
    par_c = np.prod(c_signs[core])

    Cf = f_signs @ A[0:9, :]                                         # (512, 20)
    Cp = p_signs @ A[9:16, :] + (c_signs[core] @ A[16:19, :] - A[19, :])[None, :]

    parts = {}
    for name, G in (("A", GA), ("B", GB), ("C", GC)):
        U = _subset_prods(Cp[:, G])          # (2^g, 128)
        VV = _subset_prods(Cf[:, G])         # (2^g, 512)
        V = VV[::-1]                         # complement subset: T -> 2^g-1-T
        if name == "A":
            # fold full parity: par_p(p) * par_f(f) * par_c * (-1 for d19)
            U = U * (par_p[None, :] * (-par_c))
            V = V * par_f[None, :]
        lre, lim, vtab = _pack_group(U, V)
        # NOTE: fp8 e3m4 tables (with exact per-row power-of-2 rescaling)
        # were measured at rel err 0.217 -- the subset-product expansion
        # cancels heavily (row terms ~350 summing to O(1)), amplifying
        # quantization noise ~50x. fp16 is required.
        parts[name] = tuple(x.astype(np.float16) for x in (lre, lim, vtab))

    lre, lim, vtab = parts["C"]
    tabs = {"tabC": np.ascontiguousarray(
        np.concatenate([lre, lim, vtab], axis=1))}
    for name in ("A", "B"):
        lre, lim, vtab = parts[name]
        re_chunks, im_chunks = [], []
        for k in range(2):
            sl = slice(k * 128, (k + 1) * 128)
            re_chunks += [lre[sl], vtab[sl]]
            im_chunks += [lim[sl]]
        tabs["tab%sre" % name] = np.ascontiguousarray(
            np.concatenate(re_chunks, axis=1))     # (128, 2*WRE)
        tabs["tab%sim" % name] = np.ascontiguousarray(
            np.concatenate(im_chunks, axis=1))     # (128, 2*P)
    return tabs


def _build_program():
    if "prog" in _PROGRAM_CACHE:
        return _PROGRAM_CACHE["prog"]

    from contextlib import ExitStack
    from concourse import bass, mybir

    f32 = mybir.dt.float32
    f16 = mybir.dt.float16
    mul = mybir.AluOpType.mult
    cp = mybir.ActivationFunctionType.Copy
    nc = bass.Bass()

    dram = {
        "C": nc.declare_dram_parameter("tabC", [128, 2 * P + F], f16, isOutput=False),
        "Are": nc.declare_dram_parameter("tabAre", [128, 2 * WRE], f16, isOutput=False),
        "Aim": nc.declare_dram_parameter("tabAim", [128, 2 * P], f16, isOutput=False),
        "Bre": nc.declare_dram_parameter("tabBre", [128, 2 * WRE], f16, isOutput=False),
        "Bim": nc.declare_dram_parameter("tabBim", [128, 2 * P], f16, isOutput=False),
    }
    out_dram = nc.declare_dram_parameter("out", [P, 4], f32, isOutput=True)

    es = ExitStack()
    with es:
        dma_c = es.enter_context(nc.semaphore("dma_c"))
        dma_are = es.enter_context(nc.semaphore("dma_are"))
        dma_aim = es.enter_context(nc.semaphore("dma_aim"))
        dma_bre = es.enter_context(nc.semaphore("dma_bre"))
        dma_bim = es.enter_context(nc.semaphore("dma_bim"))
        pe_sem = es.enter_context(nc.semaphore("pe_sem"))
        act_sem = es.enter_context(nc.semaphore("act_sem"))
        dve_sem = es.enter_context(nc.semaphore("dve_sem"))

        sb = {k: es.enter_context(nc.sbuf_tensor("sb_" + k, list(d.shape), f16))
              for k, d in dram.items()}
        names = ["sCre", "sCim", "sAim", "sBre",
                 "t1", "t2", "t3", "t4", "U_", "W_", "P0", "P3", "scr", "scrA"]
        wt = {n: es.enter_context(nc.sbuf_tensor(n, [P, F], f16)) for n in names}
        out_t = es.enter_context(nc.sbuf_tensor("out_t", [P, 4], f32))
        dummy = es.enter_context(nc.sbuf_tensor("actwarm", [P, 2], f32))
        pg = {}
        for g in ("A", "B", "C"):
            for comp in ("re", "im"):
                pg[g + comp] = es.enter_context(
                    nc.psum_tensor("pg" + g + comp, [P, F], f32))

        # --- Sync: C then Bre loads; final out store ---
        sync = nc.sync
        sync.dma_start(sb["C"][:, :], dram["C"][:, :]).then_inc(dma_c, 16)
        sync.dma_start(sb["Bre"][:, :], dram["Bre"][:, :]).then_inc(dma_bre, 16)
        sync.wait_ge(dve_sem, 10)
        sync.wait_ge(act_sem, 6)
        sync.dma_start(out_dram[:], out_t[:, :]).then_inc(dma_c, 16)

        # --- ACT: Are load; evictions sCre, sCim, sBre/16; reduces c0, c3 ---
        act = nc.scalar
        act.dma_start(sb["Are"][:, :], dram["Are"][:, :]).then_inc(dma_are, 16)
        # trigger walrus's activation-table load during the DMA window
        act.copy(dummy[:, 1:2], dummy[:, 0:1])
        act.wait_ge(pe_sem, 1)
        act.copy(wt["sCre"][:, :], pg["Cre"][:, :]).then_inc(act_sem, 1)
        act.wait_ge(pe_sem, 2)
        act.copy(wt["sCim"][:, :], pg["Cim"][:, :]).then_inc(act_sem, 1)
        act.wait_ge(pe_sem, 6)
        act.copy(wt["sAim"][:, :], pg["Aim"][:, :]).then_inc(act_sem, 1)
        # x1/16: U_*PBre / W_*PBre products overflow fp16 unscaled
        act.wait_ge(pe_sem, 8)
        act.mul(wt["sBre"][:, :], pg["Bre"][:, :], 0.0625).then_inc(act_sem, 1)
        act.wait_ge(dve_sem, 6)         # P0 = U_*sBre ready
        act.activation(wt["scrA"][:, :], wt["P0"][:, :], cp,
                       accum_out=out_t[:, 0:1]).then_inc(act_sem, 1)
        act.wait_ge(dve_sem, 8)         # P3 = W_*sBre ready
        act.activation(wt["scrA"][:, :], wt["P3"][:, :], cp,
                       accum_out=out_t[:, 3:4]).then_inc(act_sem, 1)

        # --- GpSimd: SWDGE loads Aim then Bim ---
        gp = nc.gpsimd
        gp.dma_start(sb["Aim"][:, :], dram["Aim"][:, :]).then_inc(dma_aim, 16)
        gp.dma_start(sb["Bim"][:, :], dram["Bim"][:, :]).then_inc(dma_bim, 16)

        # --- PE: C (2), A (4), B (4) ---
        pe = nc.tensor
        # pe_sem: pgCre 1, pgCim 2, pgAre 4, pgAim 6, pgBre 8, pgBim 10

        def mmc(comp, wait=None, thr=0):
            if wait is not None:
                pe.wait_ge(wait, thr)
            lo = 0 if comp == "re" else P
            pe.matmul(pg["C" + comp][:, :], sb["C"][:, lo:lo + P],
                      sb["C"][:, 2 * P:2 * P + F],
                      start=True, stop=True).then_inc(pe_sem, 1)

        def mm2(g, comp, k, wait=None, thr=0):
            if wait is not None:
                pe.wait_ge(wait, thr)
            if comp == "re":
                lhs = sb[g + "re"][:, k * WRE:k * WRE + P]
            else:
                lhs = sb[g + "im"][:, k * P:(k + 1) * P]
            rhs = sb[g + "re"][:, k * WRE + P:(k + 1) * WRE]
            pe.matmul(pg[g + comp][:, :], lhs, rhs,
                      start=(k == 0), stop=(k == 1)).then_inc(pe_sem, 1)

        mmc("re", dma_c, 16)
        mmc("im")
        mm2("A", "re", 0, dma_are, 16)
        mm2("A", "re", 1)
        mm2("A", "im", 0, dma_aim, 16)
        mm2("A", "im", 1)
        mm2("B", "re", 0, dma_bre, 16)
        mm2("B", "re", 1)
        mm2("B", "im", 0, dma_bim, 16)
        mm2("B", "im", 1)

        # --- DVE: combine spine ---
        v = nc.vector
        # dve_sem: t1 1, t4 2, t2 3, t3 4, U_ 5, P0 6, W_ 7, P3 8, c2 9, c1 10
        v.wait_ge(act_sem, 1)
        v.wait_ge(pe_sem, 4)
        v.tensor_mul(wt["t1"][:, :], wt["sCre"][:, :], pg["Are"][:, :]).then_inc(dve_sem, 1)
        v.wait_ge(act_sem, 2)
        v.tensor_mul(wt["t4"][:, :], wt["sCim"][:, :], pg["Are"][:, :]).then_inc(dve_sem, 1)
        # NOTE: sourcing t2 from pg["Aim"] (PSUM-direct, pe_sem>=6) while t3
        # keeps the evicted sAim reproducibly fails at runtime (NRT INTERNAL
        # error) despite compiling -- do not reintroduce that hybrid.
        v.wait_ge(act_sem, 3)
        v.tensor_mul(wt["t2"][:, :], wt["sCim"][:, :], wt["sAim"][:, :]).then_inc(dve_sem, 1)
        v.tensor_mul(wt["t3"][:, :], wt["sCre"][:, :], wt["sAim"][:, :]).then_inc(dve_sem, 1)
        v.tensor_sub(wt["U_"][:, :], wt["t1"][:, :], wt["t2"][:, :]).then_inc(dve_sem, 1)
        v.wait_ge(act_sem, 4)
        v.tensor_mul(wt["P0"][:, :], wt["U_"][:, :], wt["sBre"][:, :]).then_inc(dve_sem, 1)
        v.tensor_add(wt["W_"][:, :], wt["t3"][:, :], wt["t4"][:, :]).then_inc(dve_sem, 1)
        v.tensor_mul(wt["P3"][:, :], wt["W_"][:, :], wt["sBre"][:, :]).then_inc(dve_sem, 1)
        # out cols: 0 = sum U*sBre (ACT), 1 = sum W*PBim, 2 = sum U*PBim,
        # 3 = sum W*sBre (ACT); host: re = 16*o0 - o1, im = o2 + 16*o3.
        # (tensor_tensor_reduce does not compile on this walrus build.)
        v.wait_ge(pe_sem, 10)
        v.scalar_tensor_tensor(
            wt["scr"][:, :], wt["U_"][:, :], 1.0, pg["Bim"][:, :],
            mul, mul, accum_out=out_t[:, 2:3]).then_inc(dve_sem, 1)
        v.scalar_tensor_tensor(
            wt["scr"][:, :], wt["W_"][:, :], 1.0, pg["Bim"][:, :],
            mul, mul, accum_out=out_t[:, 1:2]).then_inc(dve_sem, 1)

    nc.finalize()
    _PROGRAM_CACHE["prog"] = nc
    return nc


def kernel(A_real, A_imag, _collect=None):
    from concourse.bass_utils import run_bass_kernel_spmd

    A = np.asarray(A_real, np.float64) + 1j * np.asarray(A_imag, np.float64)
    nc = _build_program()
    in_maps = [_build_core_tables(A, c) for c in range(N_CORES)]

    kwargs = dict(_collect or {})
    res = run_bass_kernel_spmd(nc, in_maps, core_ids=list(range(N_CORES)), **kwargs)
    if _collect is not None:
        _collect["results"] = res

    total = np.complex128(0)
    for r in res.results:
        o = np.asarray(r["out"], np.float64)
        total += (16.0 * o[:, 0] - o[:, 1]).sum() + 1j * (o[:, 2] + 16.0 * o[:, 3]).sum()

    perm = total * 2.0 * (2.0 ** (1 - N))
    ans = (perm.conjugate() * perm).real
    return np.asarray(ans, np.float32)
